# revision 42
# baseline (speedup 1.0000x reference)
"""Self-contained Trainium2 Bass kernel for batched multi-head attention
with interleaved RoPE and a block-causal mask (block size 8).

Shapes (hardcoded): x [8, 1024, 1024] f32, weights [1024, 1024] f32,
freqs_cos/sin [1024, 32] f32 -> out [8, 1024, 1024] f32.

Sharding: data-parallel over batch, one batch element per NeuronCore (8 cores).

Software-pipelined schedule: QK projection + RoPE for head-pair t+1 is
emitted interleaved into the attention steps of head-pair t, so the ScalarE
exp stream (the attention-phase bottleneck) overlaps the projection matmuls
instead of serializing after them.  Key points:
  - the two heads of a pair run their score matmuls as adjacent PE
    instructions on disjoint row groups (0-63 / 64-127) so they execute
    concurrently in the PE array; scores for both heads land in one
    [128, 2, 512] PSUM tile (two banks).
  - exp is one ACT op per (pair, bank, ktile) covering both heads.
  - block-mask multiply is one DVE op over both heads.
  - softmax denominator row is copied PSUM->SBUF on DVE (keeps the ACT
    queue pure exp), then DVE reciprocal, GPSIMD partition-broadcast,
    DVE multiply.
  - q-bank jb=1 (8 ktiles) is processed before jb=0 (4 ktiles) so the
    bufs=1 PV-accumulator stall hides under the next pair's projections.
  - V projection alternates between two PSUM pools (both idle during the
    prologue) so four chains are in flight.
"""

import os
import sys
import types

import numpy as np

B, S, D, H, HD, BS = 8, 1024, 1024, 16, 64, 8
P = 128
NT = D // P  # 8 partition tiles
NCORES = 8
HC = HD + 1  # 65: V columns per head incl. the ones column

LAST_RESULT = None  # BassKernelResults of the most recent run (for test harness)


def _install_axon_hooks():
    """Provide antenv.axon_hooks (NTFF profiling hook) when the image lacks it."""
    if "antenv.axon_hooks" in sys.modules:
        return
    try:
        import antenv
        from trn_agent_boot.trn_boot import _ntff_profile_via_ctypes

        mod = types.ModuleType("antenv.axon_hooks")
        hook = _ntff_profile_via_ctypes("/opt/axon/libaxon_pjrt.so")
        mod.get_axon_ntff_profile_hook = lambda: hook
        mod.set_axon_ntff_profile_hook = lambda h: None
        sys.modules["antenv.axon_hooks"] = mod
        antenv.axon_hooks = mod
    except Exception:
        mod = types.ModuleType("antenv.axon_hooks")
        mod.get_axon_ntff_profile_hook = lambda: None
        mod.set_axon_ntff_profile_hook = lambda h: None
        sys.modules["antenv.axon_hooks"] = mod


_NC_CACHE = {}


def _build_nc():
    """Build and compile the Bass graph (one SPMD program for all 8 cores)."""
    if "nc" in _NC_CACHE:
        return _NC_CACHE["nc"]

    import concourse.mybir as mybir
    import concourse.tile as tile
    from concourse import bacc

    BF = mybir.dt.bfloat16
    F32 = mybir.dt.float32
    MUL = mybir.AluOpType.mult
    ADD = mybir.AluOpType.add
    EXP = mybir.ActivationFunctionType.Exp

    nc = bacc.Bacc("TRN2", target_bir_lowering=False, debug=False)

    xt_d = nc.dram_tensor("xt", [D, S], BF, kind="ExternalInput")
    wq_d = nc.dram_tensor("wq", [D, D], BF, kind="ExternalInput")
    wk_d = nc.dram_tensor("wk", [D, D], BF, kind="ExternalInput")
    wv_d = nc.dram_tensor("wv", [D, D], BF, kind="ExternalInput")
    wo_d = nc.dram_tensor("wo", [D, D], BF, kind="ExternalInput")
    cos_d = nc.dram_tensor("cosf", [P, S], BF, kind="ExternalInput")
    sin_d = nc.dram_tensor("sinf", [P, S], BF, kind="ExternalInput")
    mask_d = nc.dram_tensor("mask", [P, 2 * P], BF, kind="ExternalInput")
    out_d = nc.dram_tensor("out", [S, D], F32, kind="ExternalOutput")

    scale = 1.0 / 8.0

    with tile.TileContext(nc) as tc:
        with (
            tc.tile_pool(name="big", bufs=1) as big,
            tc.tile_pool(name="proj", bufs=2, space="PSUM") as proj_ps,
            tc.tile_pool(name="stps", bufs=2, space="PSUM") as stp_ps,
            tc.tile_pool(name="otps", bufs=1, space="PSUM") as ot_ps,
            tc.tile_pool(name="work", bufs=2) as work,
            tc.tile_pool(name="ptp", bufs=3) as ptp,
        ):
            xt = [big.tile([P, S], BF, tag=f"xt{j}", name=f"xt{j}") for j in range(NT)]
            wqt = [big.tile([P, D], BF, tag=f"wq{j}", name=f"wq{j}") for j in range(NT)]
            wkt = [big.tile([P, D], BF, tag=f"wk{j}", name=f"wk{j}") for j in range(NT)]
            wvt = [big.tile([P, D], BF, tag=f"wv{j}", name=f"wv{j}") for j in range(NT)]
            wot = [big.tile([P, D], BF, tag=f"wo{j}", name=f"wo{j}") for j in range(NT)]
            qt = [big.tile([P, S], BF, tag=f"qt{t}", name=f"qt{t}") for t in range(NT)]
            kt = [big.tile([P, S], BF, tag=f"kt{t}", name=f"kt{t}") for t in range(NT)]
            vs = [big.tile([P, H * HC], BF, tag=f"vs{t}", name=f"vs{t}") for t in range(NT)]
            ot = [big.tile([P, S], BF, tag=f"ot{t}", name=f"ot{t}") for t in range(NT)]
            cosf = big.tile([P, S], BF, tag="cosf", name="cosf")
            sinf = big.tile([P, S], BF, tag="sinf", name="sinf")
            maskt = big.tile([P, 2 * P], BF, tag="mask", name="mask")
            maskv = maskt.rearrange("p (two c) -> p two c", two=2)

            # load order matters for the compute ramp: xt+wv feed the V
            # projection (needed before any attention), wq/wk next, wo last
            for j in range(NT):
                rs = slice(j * P, (j + 1) * P)
                nc.sync.dma_start(xt[j][:], xt_d[rs, :])
                nc.sync.dma_start(wvt[j][:], wv_d[rs, :])
            nc.sync.dma_start(cosf[:], cos_d[:])
            nc.sync.dma_start(sinf[:], sin_d[:])
            nc.sync.dma_start(maskt[:], mask_d[:])
            for j in range(NT):
                rs = slice(j * P, (j + 1) * P)
                nc.sync.dma_start(wqt[j][:], wq_d[rs, :])
                nc.sync.dma_start(wkt[j][:], wk_d[rs, :])
            for j in range(NT):
                rs = slice(j * P, (j + 1) * P)
                nc.sync.dma_start(wot[j][:], wo_d[rs, :])

            for t in range(NT):
                nc.vector.memset(
                    vs[t].rearrange("p (h c) -> p h c", c=HC)[:, :, HD : HD + 1], 1.0
                )

            # ---- V projection first (all of V gates the first head's PV) --
            for t in range(NT):
                cs = slice(t * P, (t + 1) * P)
                for m in range(2):
                    sl = slice(m * 512, (m + 1) * 512)
                    pv = proj_ps.tile([P, 512], F32, tag="pp", name="pv")
                    srcv = pv.rearrange("p (h c) -> p h c", c=HD)
                    for j in range(NT):
                        nc.tensor.matmul(
                            pv[:], xt[j][:, cs], wvt[j][:, sl],
                            start=(j == 0), stop=(j == NT - 1),
                        )
                    dst = vs[t].rearrange("p (h c) -> p h c", c=HC)[
                        :, m * 8 : (m + 1) * 8, 0:HD
                    ]
                    nc.vector.tensor_copy(dst, srcv)

            # ---- QK projection + rope for tile t, as a list of closures ----
            # (emitted interleaved into the previous pair's attention steps)
            def qk_chunks(t):
                chunks = []
                holder = {}

                for name, w_t, dst in (("q", wqt, qt), ("k", wkt, kt)):
                    for m in range(2):
                        sl = slice(m * 512, (m + 1) * 512)
                        for c in range(4):
                            def mm_chunk(c=c, name=name, m=m, w_t=w_t, t=t, sl=sl):
                                if c == 0:
                                    holder[(name, m)] = proj_ps.tile(
                                        [P, 512], F32, tag="pp", name=f"p{name}"
                                    )
                                pp = holder[(name, m)]
                                cs = slice(t * P, (t + 1) * P)
                                for j in (2 * c, 2 * c + 1):
                                    nc.tensor.matmul(
                                        pp[:], w_t[j][:, cs], xt[j][:, sl],
                                        start=(j == 0), stop=(j == NT - 1),
                                    )
                            chunks.append(mm_chunk)

                        def copy_chunk(name=name, m=m, dst=dst, t=t, sl=sl):
                            nc.vector.tensor_copy(dst[t][:, sl], holder[(name, m)][:])
                        chunks.append(copy_chunk)

                    # rope on the full [128, 1024] tile: rot = t*cos + swap32(t)*sin
                    def swap_chunk(name=name, dst=dst, t=t):
                        tr = work.tile([P, S], BF, tag="tr", name=f"tr{name}")
                        holder[("tr", name)] = tr
                        for b4 in range(4):
                            sblk = (b4 ^ 1) * 32
                            dblk = b4 * 32
                            nc.sync.dma_start(
                                tr[dblk : dblk + 32, :], dst[t][sblk : sblk + 32, :]
                            )
                    chunks.append(swap_chunk)

                    def rope_sin(name=name, t=t):
                        tr = holder[("tr", name)]
                        nc.vector.tensor_tensor(tr[:], tr[:], sinf[:], op=MUL)
                    chunks.append(rope_sin)

                    def rope_cos(name=name, dst=dst, t=t):
                        nc.vector.tensor_tensor(dst[t][:], dst[t][:], cosf[:], op=MUL)
                    chunks.append(rope_cos)

                    def rope_add(name=name, dst=dst, t=t):
                        tr = holder[("tr", name)]
                        nc.vector.tensor_tensor(dst[t][:], dst[t][:], tr[:], op=ADD)
                    chunks.append(rope_add)

                return chunks

            # ---- attention for head pair (2t, 2t+1), interleaving chunks --
            def attention_pair(t, chunks):
                hA, hB = 2 * t, 2 * t + 1
                otp = {}
                steps = [(1, i) for i in range(8)] + [(0, i) for i in range(4)]
                nsteps = len(steps)
                nch = len(chunks)
                ci = 0
                pend = None  # pending PV: (jb, i, pt, w, o)

                def emit_pv(jb, i, pt, w, o):
                    last_i = 7 if jb else 3
                    nc.tensor.matmul(
                        otp[(jb, 0)][:, o : o + w],
                        vs[i][:, hA * HC : (hA + 1) * HC],
                        pt[:, 0:1, 0:w],
                        start=(i == 0), stop=(i == last_i),
                    )
                    nc.tensor.matmul(
                        otp[(jb, 1)][:, o : o + w],
                        vs[i][:, hB * HC : (hB + 1) * HC],
                        pt[:, 1:2, 0:w],
                        start=(i == 0), stop=(i == last_i),
                    )

                def norm(jb, hh):
                    base = hh * HD
                    slq = slice(jb * 512, (jb + 1) * 512)
                    op = otp[(jb, hh)]
                    den = work.tile([1, 512], F32, tag="den", name="den")
                    nc.scalar.copy(den[:], op[HD : HD + 1, :])
                    rec = work.tile([1, 512], F32, tag="rec", name="rec")
                    nc.vector.reciprocal_approx_fast(rec[:], den[:])
                    bc = work.tile([HD, 512], F32, tag="bc", name="bc")
                    nc.gpsimd.partition_broadcast(bc[:], rec[:])
                    nc.vector.tensor_tensor(
                        ot[t][base : base + HD, slq], op[0:HD, :], bc[:], op=MUL
                    )

                for si, (jb, i) in enumerate(steps):
                    if i == 0:
                        otp[(jb, 0)] = ot_ps.tile([HC, 512], F32, tag="otA", name="otA")
                        otp[(jb, 1)] = ot_ps.tile([HC, 512], F32, tag="otB", name="otB")
                    q0 = max(jb * 512, i * P)
                    w = (jb + 1) * 512 - q0
                    o = q0 - jb * 512
                    stp = stp_ps.tile([P, 2, 512], F32, tag="stp", name="stp")
                    nc.tensor.matmul(
                        stp[:, 0:1, 0:w],
                        kt[t][0:HD, i * P : (i + 1) * P],
                        qt[t][0:HD, q0 : q0 + w],
                        start=True, stop=True,
                    )
                    nc.tensor.matmul(
                        stp[:, 1:2, 0:w],
                        kt[t][HD:P, i * P : (i + 1) * P],
                        qt[t][HD:P, q0 : q0 + w],
                        start=True, stop=True,
                    )
                    pt = ptp.tile([P, 2, 512], BF, tag="pt", name="pt")
                    nc.scalar.activation(
                        pt[:, :, 0:w], stp[:, :, 0:w], EXP, scale=scale
                    )
                    if i // 4 == jb:  # diagonal k-tile: block mask, both heads
                        nc.vector.tensor_tensor(
                            pt[:, :, 0:P], pt[:, :, 0:P], maskv, op=MUL
                        )
                    # interleave projection work for the next pair
                    budget = ((si + 1) * nch) // nsteps - (si * nch) // nsteps
                    for _ in range(budget):
                        chunks[ci]()
                        ci += 1
                    if pend is not None:
                        emit_pv(*pend)
                    pend = (jb, i, pt, w, o)
                    if (jb, i) == (1, 7):
                        emit_pv(*pend)
                        pend = None
                        norm(1, 0)
                        norm(1, 1)
                emit_pv(*pend)
                while ci < nch:
                    chunks[ci]()
                    ci += 1
                norm(0, 0)
                norm(0, 1)

            # prologue: QK+rope for pair 0 emitted densely
            for ch in qk_chunks(0):
                ch()
            for t in range(NT):
                attention_pair(t, qk_chunks(t + 1) if t + 1 < NT else [])

            # ---- output projection: final[s, :] = sum_i ot[i][:, s]^T wo[i]
            # s-tiles 4-7 first: they read only the jb=1 halves of ot, whose
            # pair-7 norms finish mid-pair, so the scheduler can hoist their
            # i<7 accumulation matmuls into pair-7's attention stalls.  The
            # jb=0-gated tiles (0-3) follow, overlapping the copies/DMAs.
            for st in (4, 5, 6, 7, 0, 1, 2, 3):
                cs = slice(st * P, (st + 1) * P)
                for m in range(2):
                    sl = slice(m * 512, (m + 1) * 512)
                    fp = proj_ps.tile([P, 512], F32, tag="pp", name="fp")
                    for i in range(NT):
                        nc.tensor.matmul(
                            fp[:], ot[i][:, cs], wot[i][:, sl],
                            start=(i == 0), stop=(i == NT - 1),
                        )
                    osb = work.tile([P, 512], F32, tag="osb", name="osb")
                    nc.vector.tensor_copy(osb[:], fp[:])
                    nc.sync.dma_start(out_d[cs, sl], osb[:])

    nc.compile()
    _NC_CACHE["nc"] = nc
    return nc


def _host_prep(x, wq, wk, wv, wo, freqs_cos, freqs_sin):
    import ml_dtypes

    bf16 = ml_dtypes.bfloat16

    # de-interleave RoPE pairs: permuted col c of head h maps to original
    # column h*64 + (2r if r<32 else 2(r-32)+1)
    r = np.arange(HD)
    src_local = np.where(r < 32, 2 * r, 2 * (r - 32) + 1)
    perm = (np.arange(H)[:, None] * HD + src_local[None, :]).reshape(-1)

    wq_p = np.ascontiguousarray(wq[:, perm]).astype(bf16)
    wk_p = np.ascontiguousarray(wk[:, perm]).astype(bf16)
    wv_c = np.ascontiguousarray(wv).astype(bf16)
    wo_c = np.ascontiguousarray(wo).astype(bf16)

    cos_t = np.ascontiguousarray(freqs_cos.T).astype(np.float32)  # [32, S]
    sin_t = np.ascontiguousarray(freqs_sin.T).astype(np.float32)
    cosf = np.concatenate([cos_t, cos_t, cos_t, cos_t], 0).astype(bf16)  # [128,S]
    sinf = np.concatenate([-sin_t, sin_t, -sin_t, sin_t], 0).astype(bf16)

    kq = np.arange(P)
    mask1 = ((kq[:, None] // BS) <= (kq[None, :] // BS)).astype(bf16)  # [128,128]
    mask2 = np.concatenate([mask1, mask1], 1)  # [128, 256] (both heads)

    in_maps = []
    for b in range(NCORES):
        xt = np.ascontiguousarray(x[b].T).astype(bf16)  # [D, S]
        in_maps.append(
            {
                "xt": xt,
                "wq": wq_p,
                "wk": wk_p,
                "wv": wv_c,
                "wo": wo_c,
                "cosf": cosf,
                "sinf": sinf,
                "mask": mask2,
            }
        )
    return in_maps


def kernel(x, wq, wk, wv, wo, freqs_cos, freqs_sin):
    global LAST_RESULT
    x = np.asarray(x, dtype=np.float32)
    wq = np.asarray(wq, dtype=np.float32)
    wk = np.asarray(wk, dtype=np.float32)
    wv = np.asarray(wv, dtype=np.float32)
    wo = np.asarray(wo, dtype=np.float32)
    freqs_cos = np.asarray(freqs_cos, dtype=np.float32)
    freqs_sin = np.asarray(freqs_sin, dtype=np.float32)

    trace = bool(os.environ.get("BASS_TRACE"))
    if trace:
        _install_axon_hooks()
        import concourse.bass_utils as bass_utils

        bass_utils.upload_artifacts = lambda tmpdir: tmpdir  # no-egress sandbox

    from concourse.bass_utils import run_bass_kernel_spmd

    nc = _build_nc()
    in_maps = _host_prep(x, wq, wk, wv, wo, freqs_cos, freqs_sin)
    res = run_bass_kernel_spmd(
        nc, in_maps, core_ids=list(range(NCORES)), trace=trace
    )
    LAST_RESULT = res
    out = np.stack([res.results[b]["out"] for b in range(NCORES)], 0)
    return out.astype(np.float32)


# revision 44
# speedup vs baseline: 1.2464x; 1.2464x over previous
"""Self-contained Trainium2 Bass kernel for batched multi-head attention
with interleaved RoPE and a block-causal mask (block size 8).

Shapes (hardcoded): x [8, 1024, 1024] f32, weights [1024, 1024] f32,
freqs_cos/sin [1024, 32] f32 -> out [8, 1024, 1024] f32.

Sharding: data-parallel over batch, one batch element per NeuronCore (8 cores).

Software-pipelined schedule: QK projection + RoPE for head-pair t+1 is
emitted interleaved into the attention steps of head-pair t, so the ScalarE
exp stream (the attention-phase bottleneck) overlaps the projection matmuls
instead of serializing after them.  Key points:
  - the two heads of a pair run their score matmuls as adjacent PE
    instructions on disjoint row groups (0-63 / 64-127) so they execute
    concurrently in the PE array; scores for both heads land in one
    [128, 2, 512] PSUM tile (two banks).
  - exp is one ACT op per (pair, bank, ktile) covering both heads.
  - block-mask multiply is one DVE op over both heads.
  - softmax denominator row is copied PSUM->SBUF on DVE (keeps the ACT
    queue pure exp), then DVE reciprocal, GPSIMD partition-broadcast,
    DVE multiply.
  - q-bank jb=1 (8 ktiles) is processed before jb=0 (4 ktiles) so the
    bufs=1 PV-accumulator stall hides under the next pair's projections.
  - V projection alternates between two PSUM pools (both idle during the
    prologue) so four chains are in flight.
"""

import os
import sys
import types

import numpy as np

B, S, D, H, HD, BS = 8, 1024, 1024, 16, 64, 8
P = 128
NT = D // P  # 8 partition tiles
NCORES = 8
HC = HD + 1  # 65: V columns per head incl. the ones column

LAST_RESULT = None  # BassKernelResults of the most recent run (for test harness)


def _install_axon_hooks():
    """Provide antenv.axon_hooks (NTFF profiling hook) when the image lacks it."""
    if "antenv.axon_hooks" in sys.modules:
        return
    try:
        import antenv
        from trn_agent_boot.trn_boot import _ntff_profile_via_ctypes

        mod = types.ModuleType("antenv.axon_hooks")
        hook = _ntff_profile_via_ctypes("/opt/axon/libaxon_pjrt.so")
        mod.get_axon_ntff_profile_hook = lambda: hook
        mod.set_axon_ntff_profile_hook = lambda h: None
        sys.modules["antenv.axon_hooks"] = mod
        antenv.axon_hooks = mod
    except Exception:
        mod = types.ModuleType("antenv.axon_hooks")
        mod.get_axon_ntff_profile_hook = lambda: None
        mod.set_axon_ntff_profile_hook = lambda h: None
        sys.modules["antenv.axon_hooks"] = mod


_NC_CACHE = {}


def _build_nc():
    """Build and compile the Bass graph (one SPMD program for all 8 cores)."""
    if "nc" in _NC_CACHE:
        return _NC_CACHE["nc"]

    import concourse.mybir as mybir
    import concourse.tile as tile
    from concourse import bacc

    BF = mybir.dt.bfloat16
    F32 = mybir.dt.float32
    MUL = mybir.AluOpType.mult
    ADD = mybir.AluOpType.add
    EXP = mybir.ActivationFunctionType.Exp

    nc = bacc.Bacc("TRN2", target_bir_lowering=False, debug=False)

    xt_d = nc.dram_tensor("xt", [D, S], BF, kind="ExternalInput")
    wq_d = nc.dram_tensor("wq", [D, D], BF, kind="ExternalInput")
    wk_d = nc.dram_tensor("wk", [D, D], BF, kind="ExternalInput")
    wv_d = nc.dram_tensor("wv", [D, D], BF, kind="ExternalInput")
    wo_d = nc.dram_tensor("wo", [D, D], BF, kind="ExternalInput")
    cos_d = nc.dram_tensor("cosf", [P, S], BF, kind="ExternalInput")
    sin_d = nc.dram_tensor("sinf", [P, S], BF, kind="ExternalInput")
    mask_d = nc.dram_tensor("mask", [P, 2 * P], BF, kind="ExternalInput")
    out_d = nc.dram_tensor("out", [S, D], F32, kind="ExternalOutput")

    scale = 1.0 / 8.0

    with tile.TileContext(nc) as tc:
        with (
            tc.tile_pool(name="big", bufs=1) as big,
            tc.tile_pool(name="proj", bufs=2, space="PSUM") as proj_ps,
            tc.tile_pool(name="stps", bufs=2, space="PSUM") as stp_ps,
            tc.tile_pool(name="otps", bufs=1, space="PSUM") as ot_ps,
            tc.tile_pool(name="work", bufs=2) as work,
            tc.tile_pool(name="ptp", bufs=3) as ptp,
        ):
            xt = [big.tile([P, S], BF, tag=f"xt{j}", name=f"xt{j}") for j in range(NT)]
            wqt = [big.tile([P, D], BF, tag=f"wq{j}", name=f"wq{j}") for j in range(NT)]
            wkt = [big.tile([P, D], BF, tag=f"wk{j}", name=f"wk{j}") for j in range(NT)]
            wvt = [big.tile([P, D], BF, tag=f"wv{j}", name=f"wv{j}") for j in range(NT)]
            wot = [big.tile([P, D], BF, tag=f"wo{j}", name=f"wo{j}") for j in range(NT)]
            qt = [big.tile([P, S], BF, tag=f"qt{t}", name=f"qt{t}") for t in range(NT)]
            kt = [big.tile([P, S], BF, tag=f"kt{t}", name=f"kt{t}") for t in range(NT)]
            vs = [big.tile([P, H * HC], BF, tag=f"vs{t}", name=f"vs{t}") for t in range(NT)]
            ot = [big.tile([P, S], BF, tag=f"ot{t}", name=f"ot{t}") for t in range(NT)]
            cosf = big.tile([P, S], BF, tag="cosf", name="cosf")
            sinf = big.tile([P, S], BF, tag="sinf", name="sinf")
            maskt = big.tile([P, 2 * P], BF, tag="mask", name="mask")
            maskv = maskt.rearrange("p (two c) -> p two c", two=2)

            # load order matters for the compute ramp: xt+wv feed the V
            # projection (needed before any attention), wq/wk next, wo last
            for j in range(NT):
                rs = slice(j * P, (j + 1) * P)
                nc.sync.dma_start(xt[j][:], xt_d[rs, :])
                nc.sync.dma_start(wvt[j][:], wv_d[rs, :])
            for j in range(NT):
                rs = slice(j * P, (j + 1) * P)
                nc.sync.dma_start(wqt[j][:], wq_d[rs, :])
                nc.sync.dma_start(wkt[j][:], wk_d[rs, :])
            nc.sync.dma_start(cosf[:], cos_d[:])
            nc.sync.dma_start(sinf[:], sin_d[:])
            nc.sync.dma_start(maskt[:], mask_d[:])
            for j in range(NT):
                rs = slice(j * P, (j + 1) * P)
                nc.sync.dma_start(wot[j][:], wo_d[rs, :])

            for t in range(NT):
                nc.vector.memset(
                    vs[t].rearrange("p (h c) -> p h c", c=HC)[:, :, HD : HD + 1], 1.0
                )

            # ---- V projection first (all of V gates the first head's PV) --
            # j-major: the stationary xt[j] slice serves both 512-wide moving
            # banks (halves LDWEIGHTS count).  Chains alternate between one
            # stp-pool tile (holds both banks, idle during the prologue) and
            # a pair of pp-pool tiles so the PSUM->SBUF copies stay hidden.
            for t in range(NT):
                cs = slice(t * P, (t + 1) * P)
                if t % 2 == 0:
                    both = stp_ps.tile([P, 2, 512], F32, tag="stp", name="pvs")
                    pv = [both[:, 0:1, :], both[:, 1:2, :]]
                    srcv = [
                        pv[m].rearrange("p one (h c) -> p (one h) c", c=HD)
                        for m in range(2)
                    ]
                else:
                    pv = [
                        proj_ps.tile([P, 512], F32, tag="pp", name="pv")
                        for _ in range(2)
                    ]
                    srcv = [pv[m].rearrange("p (h c) -> p h c", c=HD)
                            for m in range(2)]
                for j in range(NT):
                    for m in range(2):
                        nc.tensor.matmul(
                            pv[m][:], xt[j][:, cs],
                            wvt[j][:, m * 512 : (m + 1) * 512],
                            start=(j == 0), stop=(j == NT - 1),
                        )
                for m in range(2):
                    dst = vs[t].rearrange("p (h c) -> p h c", c=HC)[
                        :, m * 8 : (m + 1) * 8, 0:HD
                    ]
                    nc.vector.tensor_copy(dst, srcv[m])

            # ---- QK projection + rope for tile t, as a list of closures ----
            # (emitted interleaved into the previous pair's attention steps)
            def qk_chunks(t):
                chunks = []
                holder = {}

                for name, w_t, dst in (("q", wqt, qt), ("k", wkt, kt)):
                    for c in range(4):
                        def mm_chunk(c=c, name=name, w_t=w_t, t=t):
                            if c == 0:
                                for m in range(2):
                                    holder[(name, m)] = proj_ps.tile(
                                        [P, 512], F32, tag="pp", name=f"p{name}{m}"
                                    )
                            cs = slice(t * P, (t + 1) * P)
                            for j in (2 * c, 2 * c + 1):
                                for m in range(2):
                                    nc.tensor.matmul(
                                        holder[(name, m)][:], w_t[j][:, cs],
                                        xt[j][:, m * 512 : (m + 1) * 512],
                                        start=(j == 0), stop=(j == NT - 1),
                                    )
                        chunks.append(mm_chunk)

                    for m in range(2):
                        def copy_chunk(name=name, m=m, dst=dst, t=t):
                            nc.vector.tensor_copy(
                                dst[t][:, m * 512 : (m + 1) * 512],
                                holder[(name, m)][:],
                            )
                        chunks.append(copy_chunk)

                    # rope on the full [128, 1024] tile: rot = t*cos + swap32(t)*sin
                    def swap_chunk(name=name, dst=dst, t=t):
                        tr = work.tile([P, S], BF, tag="tr", name=f"tr{name}")
                        holder[("tr", name)] = tr
                        for b4 in range(4):
                            sblk = (b4 ^ 1) * 32
                            dblk = b4 * 32
                            nc.sync.dma_start(
                                tr[dblk : dblk + 32, :], dst[t][sblk : sblk + 32, :]
                            )
                    chunks.append(swap_chunk)

                    def rope_sin(name=name, t=t):
                        tr = holder[("tr", name)]
                        nc.vector.tensor_tensor(tr[:], tr[:], sinf[:], op=MUL)
                    chunks.append(rope_sin)

                    def rope_cos(name=name, dst=dst, t=t):
                        nc.vector.tensor_tensor(dst[t][:], dst[t][:], cosf[:], op=MUL)
                    chunks.append(rope_cos)

                    def rope_add(name=name, dst=dst, t=t):
                        tr = holder[("tr", name)]
                        nc.vector.tensor_tensor(dst[t][:], dst[t][:], tr[:], op=ADD)
                    chunks.append(rope_add)

                return chunks

            # ---- attention for head pair (2t, 2t+1), interleaving chunks --
            def attention_pair(t, chunks):
                hA, hB = 2 * t, 2 * t + 1
                otp = {}
                steps = [(1, i) for i in range(8)] + [(0, i) for i in range(4)]
                nsteps = len(steps)
                nch = len(chunks)
                ci = 0
                pend = None  # pending PV: (jb, i, pt, w, o)

                def emit_pv(jb, i, pt, w, o):
                    last_i = 7 if jb else 3
                    nc.tensor.matmul(
                        otp[(jb, 0)][:, o : o + w],
                        vs[i][:, hA * HC : (hA + 1) * HC],
                        pt[:, 0:1, 0:w],
                        start=(i == 0), stop=(i == last_i),
                    )
                    nc.tensor.matmul(
                        otp[(jb, 1)][:, o : o + w],
                        vs[i][:, hB * HC : (hB + 1) * HC],
                        pt[:, 1:2, 0:w],
                        start=(i == 0), stop=(i == last_i),
                    )

                def norm(jb, hh):
                    base = hh * HD
                    slq = slice(jb * 512, (jb + 1) * 512)
                    op = otp[(jb, hh)]
                    den = work.tile([1, 512], F32, tag="den", name="den")
                    nc.scalar.copy(den[:], op[HD : HD + 1, :])
                    rec = work.tile([1, 512], F32, tag="rec", name="rec")
                    nc.vector.reciprocal_approx_fast(rec[:], den[:])
                    bc = work.tile([HD, 512], F32, tag="bc", name="bc")
                    nc.gpsimd.partition_broadcast(bc[:], rec[:])
                    nc.vector.tensor_tensor(
                        ot[t][base : base + HD, slq], op[0:HD, :], bc[:], op=MUL
                    )

                for si, (jb, i) in enumerate(steps):
                    if i == 0:
                        otp[(jb, 0)] = ot_ps.tile([HC, 512], F32, tag="otA", name="otA")
                        otp[(jb, 1)] = ot_ps.tile([HC, 512], F32, tag="otB", name="otB")
                    q0 = max(jb * 512, i * P)
                    w = (jb + 1) * 512 - q0
                    o = q0 - jb * 512
                    stp = stp_ps.tile([P, 2, 512], F32, tag="stp", name="stp")
                    nc.tensor.matmul(
                        stp[:, 0:1, 0:w],
                        kt[t][0:HD, i * P : (i + 1) * P],
                        qt[t][0:HD, q0 : q0 + w],
                        start=True, stop=True,
                    )
                    nc.tensor.matmul(
                        stp[:, 1:2, 0:w],
                        kt[t][HD:P, i * P : (i + 1) * P],
                        qt[t][HD:P, q0 : q0 + w],
                        start=True, stop=True,
                    )
                    pt = ptp.tile([P, 2, 512], BF, tag="pt", name="pt")
                    nc.scalar.activation(
                        pt[:, :, 0:w], stp[:, :, 0:w], EXP, scale=scale
                    )
                    if i // 4 == jb:  # diagonal k-tile: block mask, both heads
                        nc.vector.tensor_tensor(
                            pt[:, :, 0:P], pt[:, :, 0:P], maskv, op=MUL
                        )
                    # interleave projection work for the next pair
                    budget = ((si + 1) * nch) // nsteps - (si * nch) // nsteps
                    for _ in range(budget):
                        chunks[ci]()
                        ci += 1
                    if pend is not None:
                        emit_pv(*pend)
                    pend = (jb, i, pt, w, o)
                    if (jb, i) == (1, 7):
                        emit_pv(*pend)
                        pend = None
                        norm(1, 0)
                        norm(1, 1)
                emit_pv(*pend)
                while ci < nch:
                    chunks[ci]()
                    ci += 1
                norm(0, 0)
                norm(0, 1)

            # prologue: QK+rope for pair 0 emitted densely
            for ch in qk_chunks(0):
                ch()
            for t in range(NT):
                attention_pair(t, qk_chunks(t + 1) if t + 1 < NT else [])

            # ---- output projection: final[s, :] = sum_i ot[i][:, s]^T wo[i]
            # s-tiles 4-7 first: they read only the jb=1 halves of ot, whose
            # pair-7 norms finish mid-pair, so the scheduler can hoist their
            # i<7 accumulation matmuls into pair-7's attention stalls.  The
            # jb=0-gated tiles (0-3) follow, overlapping the copies/DMAs.
            for st in (4, 5, 6, 7, 0, 1, 2, 3):
                cs = slice(st * P, (st + 1) * P)
                for m in range(2):
                    sl = slice(m * 512, (m + 1) * 512)
                    fp = proj_ps.tile([P, 512], F32, tag="pp", name="fp")
                    for i in range(NT):
                        nc.tensor.matmul(
                            fp[:], ot[i][:, cs], wot[i][:, sl],
                            start=(i == 0), stop=(i == NT - 1),
                        )
                    osb = work.tile([P, 512], F32, tag="osb", name="osb")
                    nc.vector.tensor_copy(osb[:], fp[:])
                    nc.sync.dma_start(out_d[cs, sl], osb[:])

    nc.compile()
    _NC_CACHE["nc"] = nc
    return nc


def _host_prep(x, wq, wk, wv, wo, freqs_cos, freqs_sin):
    import ml_dtypes

    bf16 = ml_dtypes.bfloat16

    # de-interleave RoPE pairs: permuted col c of head h maps to original
    # column h*64 + (2r if r<32 else 2(r-32)+1)
    r = np.arange(HD)
    src_local = np.where(r < 32, 2 * r, 2 * (r - 32) + 1)
    perm = (np.arange(H)[:, None] * HD + src_local[None, :]).reshape(-1)

    wq_p = np.ascontiguousarray(wq[:, perm]).astype(bf16)
    wk_p = np.ascontiguousarray(wk[:, perm]).astype(bf16)
    wv_c = np.ascontiguousarray(wv).astype(bf16)
    wo_c = np.ascontiguousarray(wo).astype(bf16)

    cos_t = np.ascontiguousarray(freqs_cos.T).astype(np.float32)  # [32, S]
    sin_t = np.ascontiguousarray(freqs_sin.T).astype(np.float32)
    cosf = np.concatenate([cos_t, cos_t, cos_t, cos_t], 0).astype(bf16)  # [128,S]
    sinf = np.concatenate([-sin_t, sin_t, -sin_t, sin_t], 0).astype(bf16)

    kq = np.arange(P)
    mask1 = ((kq[:, None] // BS) <= (kq[None, :] // BS)).astype(bf16)  # [128,128]
    mask2 = np.concatenate([mask1, mask1], 1)  # [128, 256] (both heads)

    in_maps = []
    for b in range(NCORES):
        xt = np.ascontiguousarray(x[b].T).astype(bf16)  # [D, S]
        in_maps.append(
            {
                "xt": xt,
                "wq": wq_p,
                "wk": wk_p,
                "wv": wv_c,
                "wo": wo_c,
                "cosf": cosf,
                "sinf": sinf,
                "mask": mask2,
            }
        )
    return in_maps


def kernel(x, wq, wk, wv, wo, freqs_cos, freqs_sin):
    global LAST_RESULT
    x = np.asarray(x, dtype=np.float32)
    wq = np.asarray(wq, dtype=np.float32)
    wk = np.asarray(wk, dtype=np.float32)
    wv = np.asarray(wv, dtype=np.float32)
    wo = np.asarray(wo, dtype=np.float32)
    freqs_cos = np.asarray(freqs_cos, dtype=np.float32)
    freqs_sin = np.asarray(freqs_sin, dtype=np.float32)

    trace = bool(os.environ.get("BASS_TRACE"))
    if trace:
        _install_axon_hooks()
        import concourse.bass_utils as bass_utils

        bass_utils.upload_artifacts = lambda tmpdir: tmpdir  # no-egress sandbox

    from concourse.bass_utils import run_bass_kernel_spmd

    nc = _build_nc()
    in_maps = _host_prep(x, wq, wk, wv, wo, freqs_cos, freqs_sin)
    res = run_bass_kernel_spmd(
        nc, in_maps, core_ids=list(range(NCORES)), trace=trace
    )
    LAST_RESULT = res
    out = np.stack([res.results[b]["out"] for b in range(NCORES)], 0)
    return out.astype(np.float32)
